# revision 19
# baseline (speedup 1.0000x reference)
"""ContrastiveHead loss kernel for 8 Trainium2 NeuronCores.

Strategy: data-parallel shard B across the 8 cores. Each core runs the
3-layer MLP for its 2*B/8 = 1024 rows in TWO row-groups of 512 so the
all-gather of group 0's normalized fp8 features (Shared-HBM output
collective) overlaps group 1's MLP, and the similarity/exp work for
group-0 columns overlaps the group-1 gather.

Layouts: activations ride transposed ([features-on-partitions, rows-on-
free]); the host pre-transposes the input shard and pre-tiles the
weights into [n_tile][pk, k_tile, jn] slabs. Hidden layers run fp8
DoubleRow; features are normalized then cast to fp8e4m3 for the gather
and the sim matmul (errors in pos cancel against the partner term in
lse, measured rel err ~1e-4).

logsumexp uses the constant bound max=1.0 (normalized rows: sim <= 1):
lse = 1/T + log(sum_j exp((S_ij-1)/T)). The self term is excluded by
subtracting exp((S_ii-1)/T), with S_ii recomputed from the same fp8
bytes the gathered matmul reads, so the cancellation is bit-exact.
"""

import os
import sys

for _p in ("/opt/trn_rl_repo",):
    if os.path.isdir(_p) and _p not in sys.path:
        sys.path.append(_p)

import ml_dtypes
import numpy as np

import concourse.bass as bass
import concourse.mybir as mybir
import concourse.tile as tile
from concourse import bacc
from concourse.bass_utils import run_bass_kernel_spmd
from concourse.masks import make_identity

BF16 = ml_dtypes.bfloat16
F32 = mybir.dt.float32
BF = mybir.dt.bfloat16
F8 = mybir.dt.float8e4
FP8 = mybir.dt.np(F8)

B, D, H, E = 4096, 2048, 2048, 128
T = 0.07
SCALE = float(1.0 / T)
NCORES = 8
BS = B // NCORES          # rows per view per core (512)
M = 2 * BS                # local feature rows (1024)
KT = D // 128             # 16 contraction tiles for D/H
NT = H // 128             # 16 output-feature tiles for hidden layers
MT = M // 128             # 8 local row tiles
NG = NCORES * M           # 8192 gathered rows
NGRP = 2                  # row groups per core (512 rows each)
GR = M // NGRP            # rows per group
SKIP = set(os.environ.get("KERNEL_SKIP", "").split(",")) - {""}
CC_ADDR_SPACE = os.environ.get("KERNEL_CC_ADDR", "Shared")
EXPCHUNK = int(os.environ.get("KERNEL_EXPCHUNK", "1"))


def _build():
    nc = bacc.Bacc(num_devices=NCORES)

    x = nc.dram_tensor("x", [128, KT, M], F8, kind="ExternalInput")
    w0 = nc.dram_tensor("w0", [NT, 128, KT, 128], F8, kind="ExternalInput")
    w1 = nc.dram_tensor("w1", [NT, 128, KT, 128], F8, kind="ExternalInput")
    w2 = nc.dram_tensor("w2", [128, KT, 128], BF, kind="ExternalInput")
    b0 = nc.dram_tensor("b0", [128, NT], F32, kind="ExternalInput")
    b1 = nc.dram_tensor("b1", [128, NT], F32, kind="ExternalInput")
    b2 = nc.dram_tensor("b2", [128, 1], F32, kind="ExternalInput")
    out = nc.dram_tensor("out", [128, MT], F32, kind="ExternalOutput")

    AF = mybir.ActivationFunctionType

    with tile.TileContext(nc) as tc:
        with (
            tc.tile_pool(name="singles", bufs=1) as singles,
            tc.tile_pool(name="small", bufs=4) as small,
            tc.tile_pool(name="esc", bufs=3) as esc,
            tc.tile_pool(name="pmm", bufs=2, space="PSUM") as pmm,
            tc.tile_pool(name="psim", bufs=2, space="PSUM") as psim,
            tc.tile_pool(name="psmall", bufs=2, space="PSUM") as psmall,
            tc.tile_pool(name="dram", bufs=1, space="DRAM") as dram,
        ):
            # ---- constants ----
            ident = singles.tile([128, 128], F32)
            make_identity(nc, ident)
            b0s = singles.tile([128, NT], F32)
            b1s = singles.tile([128, NT], F32)
            b2s = singles.tile([128, 1], F32)
            nc.sync.dma_start(out=b0s, in_=b0[:, :])
            nc.sync.dma_start(out=b1s, in_=b1[:, :])
            nc.sync.dma_start(out=b2s, in_=b2[:, :])

            # ---- activations + resident weights ----
            a_x = singles.tile([128, KT, M], F8)
            for g in range(NGRP):
                gs = slice(g * GR, (g + 1) * GR)
                for tk in range(KT):
                    nc.sync.dma_start(out=a_x[:, tk, gs], in_=x[:, tk, gs])
                if g == 0:
                    wall0 = singles.tile([128, NT, KT, 128], F8)
                    for tn in range(NT):
                        nc.sync.dma_start(out=wall0[:, tn], in_=w0[tn])
            wall1 = singles.tile([128, NT, KT, 128], F8)
            for tn in range(NT):
                nc.sync.dma_start(out=wall1[:, tn], in_=w1[tn])
            wsl2 = singles.tile([128, KT, 128], BF)
            nc.sync.dma_start(out=wsl2, in_=w2[:, :, :])

            a_h0 = singles.tile([128, NT, M], F8)
            a_h1 = singles.tile([128, NT, M], BF)
            eT = singles.tile([128, M], F32)
            sq = singles.tile([128, M], F32)
            rnorm = singles.tile([128, M], F32)
            fT8 = singles.tile([128, M], F8)
            FT = singles.tile([128, NG], F8)
            dself_all = singles.tile([128, MT], F32)
            dpos_all = singles.tile([128, MT], F32)
            sums_all = singles.tile([128, MT, NGRP * (NCORES // EXPCHUNK)], F32)
            ones = singles.tile([128, 128], F32)
            nc.vector.memset(ones, 1.0)
            nbias = singles.tile([128, 1], F32)
            nc.vector.memset(nbias, -SCALE)
            pbias = singles.tile([128, 1], F32)
            nc.vector.memset(pbias, SCALE)

            def hidden_layer(src, dst, wall, bias_s, func, msl):
                """One fp8 DoubleRow layer for the 512-row slice msl."""
                for tn in range(NT):
                    ps = pmm.tile([128, 512], F32, tag="mm")
                    for tk in range(0, KT, 2):
                        nc.tensor.matmul(
                            ps,
                            lhsT=wall[:, tn, tk : tk + 2, :],
                            rhs=src[:, tk : tk + 2, msl],
                            start=(tk == 0),
                            stop=(tk == KT - 2),
                            perf_mode=mybir.MatmulPerfMode.DoubleRow,
                        )
                    nc.scalar.activation(
                        out=dst[:, tn, msl], in_=ps, func=func,
                        bias=bias_s[:, tn : tn + 1], scale=1.0,
                    )

            def diag(m, pm, dst):
                """dst[:, m] = diag of fT8_m^T @ fT8_pm (bit-exact fp8 path)."""
                ps_d = psmall.tile([128, 128], F32, tag="ps_small")
                nc.tensor.matmul(
                    ps_d,
                    lhsT=fT8[:, m * 128 : (m + 1) * 128],
                    rhs=fT8[:, pm * 128 : (pm + 1) * 128],
                    start=True, stop=True,
                )
                dsc = small.tile([128, 128], F32, tag="dscratch")
                nc.vector.tensor_mul(dsc, ps_d, ident)
                nc.vector.reduce_sum(
                    dst[:, m : m + 1], dsc, axis=mybir.AxisListType.X
                )

            cc_outs = []
            for g in range(NGRP):
                msl = slice(g * GR, (g + 1) * GR)
                hidden_layer(a_x, a_h0, wall0, b0s, AF.Relu, msl)
                hidden_layer(a_h0, a_h1, wall1, b1s, AF.Identity, msl)

                # layer 2 -> eT[:, msl] fp32
                ps2 = pmm.tile([128, 512], F32, tag="mm")
                for tk in range(KT):
                    nc.tensor.matmul(
                        ps2, lhsT=wsl2[:, tk, :], rhs=a_h1[:, tk, msl],
                        start=(tk == 0), stop=(tk == KT - 1),
                    )
                nc.scalar.activation(
                    out=eT[:, msl], in_=ps2, func=AF.Identity,
                    bias=b2s[:, 0:1], scale=1.0,
                )

                # normalize -> fT8[:, msl]
                nc.vector.tensor_mul(sq[:, msl], eT[:, msl], eT[:, msl])
                psn = pmm.tile([128, 512], F32, tag="mm")
                nc.tensor.matmul(
                    psn, lhsT=ones, rhs=sq[:, msl], start=True, stop=True
                )
                nc.scalar.activation(
                    out=rnorm[:, msl], in_=psn, func=AF.Sqrt, scale=1.0
                )
                nc.vector.reciprocal(out=rnorm[:, msl], in_=rnorm[:, msl])
                nc.vector.tensor_mul(fT8[:, msl], eT[:, msl], rnorm[:, msl])

                # self-similarity diagonals for this group's row tiles
                for m in range(g * MT // NGRP, (g + 1) * MT // NGRP):
                    diag(m, m, dself_all)

                # all-gather this group's normalized features (fp8)
                cc_in = dram.tile([128, GR], F8, name=f"cc_in{g}")
                cc_out = dram.tile(
                    [NCORES * 128, GR], F8, addr_space=CC_ADDR_SPACE,
                    name=f"cc_out{g}",
                )
                nc.sync.dma_start(out=cc_in, in_=fT8[:, msl])
                if "collective" in SKIP:
                    for r in range(NCORES):
                        nc.sync.dma_start(
                            out=cc_out[r * 128 : (r + 1) * 128, :], in_=cc_in[:, :]
                        )
                else:
                    nc.gpsimd.collective_compute(
                        "AllGather",
                        mybir.AluOpType.bypass,
                        replica_groups=[list(range(NCORES))],
                        ins=[cc_in.opt()],
                        outs=[cc_out.opt()],
                    )
                cc_outs.append(cc_out)
                for r in range(NCORES):
                    nc.sync.dma_start(
                        out=FT[:, r * M + g * GR : r * M + (g + 1) * GR],
                        in_=cc_out[r * 128 : (r + 1) * 128, :],
                    )

            # partner-similarity diagonals (pairs cross the two groups)
            for m in range(MT):
                diag(m, (m + MT // 2) % MT, dpos_all)

            # ---- sim + exp-sum: group-0 columns first, then group-1 ----
            outv = singles.tile([128, MT], F32)
            CPG = NCORES // EXPCHUNK  # 1024-col chunks per column group
            for g in range(NGRP):
                for m in range(MT):
                    lhs = fT8[:, m * 128 : (m + 1) * 128]
                    for j in range(CPG):
                        ps = psim.tile([128, EXPCHUNK * 512], F32, tag="mm2")
                        for h in range(EXPCHUNK):
                            r = j * EXPCHUNK + h
                            nc.tensor.matmul(
                                ps[:, h * 512 : (h + 1) * 512], lhsT=lhs,
                                rhs=FT[:, r * M + g * GR : r * M + g * GR + 512],
                                start=True, stop=True,
                            )
                        escr = esc.tile([128, EXPCHUNK * 512], BF, tag="escr")
                        nc.scalar.activation(
                            out=escr, in_=ps, func=AF.Exp, scale=SCALE, bias=nbias,
                            accum_out=sums_all[:, m, g * CPG + j : g * CPG + j + 1],
                        )

            # ---- batched epilogue ----
            stot_all = singles.tile([128, MT], F32)
            for m in range(MT):
                nc.vector.reduce_sum(
                    stot_all[:, m : m + 1], sums_all[:, m, : 2 * CPG],
                    axis=mybir.AxisListType.X,
                )
            eself = small.tile([128, MT], F32, tag="eself")
            nc.scalar.activation(
                out=eself, in_=dself_all, func=AF.Exp, scale=SCALE, bias=nbias
            )
            sexcl = small.tile([128, MT], F32, tag="sexcl")
            nc.vector.tensor_sub(sexcl, stot_all, eself)
            lsep = small.tile([128, MT], F32, tag="lsep")
            nc.scalar.activation(out=lsep, in_=sexcl, func=AF.Ln, scale=1.0)
            post = small.tile([128, MT], F32, tag="post")
            nc.scalar.activation(
                out=post, in_=dpos_all, func=AF.Identity, scale=-SCALE, bias=pbias
            )
            nc.vector.tensor_add(outv, lsep, post)

            nc.sync.dma_start(out=out[:, :], in_=outv)

    nc.finalize()
    return nc


_NC_CACHE = None


def _get_nc():
    global _NC_CACHE
    if _NC_CACHE is None:
        _NC_CACHE = _build()
    return _NC_CACHE


def _prep_w(W, ntiles, dt=BF16):
    K = W.shape[0]
    kt = K // 128
    arr = W.reshape(kt, 128, ntiles, 128).transpose(2, 1, 0, 3)
    return np.ascontiguousarray(arr.astype(dt))


def _prep_b(b, ntiles):
    return np.ascontiguousarray(
        np.asarray(b, np.float32).reshape(ntiles, 128).T
    )


def kernel(input1, input2, W0, b0, W1, b1, W2, b2):
    input1 = np.asarray(input1, np.float32)
    input2 = np.asarray(input2, np.float32)
    w0p = _prep_w(np.asarray(W0, np.float32), NT, FP8)
    w1p = _prep_w(np.asarray(W1, np.float32), NT, FP8)
    w2p = _prep_w(np.asarray(W2, np.float32), 1)[0]
    b0p = _prep_b(b0, NT)
    b1p = _prep_b(b1, NT)
    b2p = np.ascontiguousarray(np.asarray(b2, np.float32).reshape(128, 1))

    in_maps = []
    for r in range(NCORES):
        xr = np.concatenate(
            [input1[r * BS : (r + 1) * BS], input2[r * BS : (r + 1) * BS]], axis=0
        )
        xp = np.ascontiguousarray(
            xr.reshape(M, KT, 128).transpose(2, 1, 0).astype(FP8)
        )
        in_maps.append(
            {
                "x": xp, "w0": w0p, "w1": w1p, "w2": w2p,
                "b0": b0p, "b1": b1p, "b2": b2p,
            }
        )

    nc = _get_nc()
    res = run_bass_kernel_spmd(
        nc,
        in_maps,
        core_ids=list(range(NCORES)),
        trace=bool(int(os.environ.get("KERNEL_TRACE", "0"))),
    )
    total = np.float64(0.0)
    for r in range(NCORES):
        total += np.asarray(res.results[r]["out"], np.float64).sum()
    loss = np.float32(total / (2 * B))
    if res.exec_time_ns is not None:
        kernel.last_exec_time_ns = res.exec_time_ns
    return np.asarray(loss, np.float32)


kernel.last_exec_time_ns = None


# revision 22
# speedup vs baseline: 1.1839x; 1.1839x over previous
"""ContrastiveHead loss kernel for 8 Trainium2 NeuronCores.

Strategy: data-parallel shard B across the 8 cores. Each core runs the
3-layer MLP for its 2*B/8 = 1024 rows in TWO row-groups of 512 so the
all-gather of group 0's normalized fp8 features (Shared-HBM output
collective) overlaps group 1's MLP, and the similarity/exp work for
group-0 columns overlaps the group-1 gather.

Layouts: activations ride transposed ([features-on-partitions, rows-on-
free]); the host pre-transposes the input shard and pre-tiles the
weights into [n_tile][pk, k_tile, jn] slabs. Hidden layers run fp8
DoubleRow; features are normalized then cast to fp8e4m3 for the gather
and the sim matmul (errors in pos cancel against the partner term in
lse, measured rel err ~1e-4).

logsumexp uses the constant bound max=1.0 (normalized rows: sim <= 1):
lse = 1/T + log(sum_j exp((S_ij-1)/T)). The self term is excluded by
subtracting exp((S_ii-1)/T), with S_ii recomputed from the same fp8
bytes the gathered matmul reads, so the cancellation is bit-exact.
"""

import os
import sys

for _p in ("/opt/trn_rl_repo",):
    if os.path.isdir(_p) and _p not in sys.path:
        sys.path.append(_p)

import ml_dtypes
import numpy as np

import concourse.bass as bass
import concourse.mybir as mybir
import concourse.tile as tile
from concourse import bacc
from concourse.bass_utils import run_bass_kernel_spmd
from concourse.masks import make_identity

BF16 = ml_dtypes.bfloat16
F32 = mybir.dt.float32
BF = mybir.dt.bfloat16
F8 = mybir.dt.float8e4
FP8 = mybir.dt.np(F8)

B, D, H, E = 4096, 2048, 2048, 128
T = 0.07
SCALE = float(1.0 / T)
NCORES = 8
BS = B // NCORES          # rows per view per core (512)
M = 2 * BS                # local feature rows (1024)
KT = D // 128             # 16 contraction tiles for D/H
NT = H // 128             # 16 output-feature tiles for hidden layers
MT = M // 128             # 8 local row tiles
NG = NCORES * M           # 8192 gathered rows
NGRP = 2                  # row groups per core (512 rows each)
GR = M // NGRP            # rows per group
SKIP = set(os.environ.get("KERNEL_SKIP", "").split(",")) - {""}
CC_ADDR_SPACE = os.environ.get("KERNEL_CC_ADDR", "Shared")
EXPCHUNK = int(os.environ.get("KERNEL_EXPCHUNK", "1"))
LDW = os.environ.get("KERNEL_LDW", "0") == "1"


def _build():
    nc = bacc.Bacc(num_devices=NCORES)

    x = nc.dram_tensor("x", [128, KT, M], F8, kind="ExternalInput")
    w0 = nc.dram_tensor("w0", [NT, 128, KT, 128], F8, kind="ExternalInput")
    w1 = nc.dram_tensor("w1", [NT, 128, KT, 128], F8, kind="ExternalInput")
    w2 = nc.dram_tensor("w2", [128, KT, 128], BF, kind="ExternalInput")
    b0 = nc.dram_tensor("b0", [128, NT], F32, kind="ExternalInput")
    b1 = nc.dram_tensor("b1", [128, NT], F32, kind="ExternalInput")
    b2 = nc.dram_tensor("b2", [128, 1], F32, kind="ExternalInput")
    out = nc.dram_tensor("out", [128, MT], F32, kind="ExternalOutput")

    AF = mybir.ActivationFunctionType

    with tile.TileContext(nc) as tc:
        with (
            tc.tile_pool(name="singles", bufs=1) as singles,
            tc.tile_pool(name="small", bufs=4) as small,
            tc.tile_pool(name="esc", bufs=3) as esc,
            tc.tile_pool(name="pmm", bufs=2, space="PSUM") as pmm,
            tc.tile_pool(name="psim", bufs=2, space="PSUM") as psim,
            tc.tile_pool(name="psmall", bufs=2, space="PSUM") as psmall,
            tc.tile_pool(name="dram", bufs=1, space="DRAM") as dram,
        ):
            # ---- constants ----
            ident = singles.tile([128, 128], F32)
            make_identity(nc, ident)
            b0s = singles.tile([128, NT], F32)
            b1s = singles.tile([128, NT], F32)
            b2s = singles.tile([128, 1], F32)
            nc.sync.dma_start(out=b0s, in_=b0[:, :])
            nc.sync.dma_start(out=b1s, in_=b1[:, :])
            nc.sync.dma_start(out=b2s, in_=b2[:, :])

            # ---- activations + resident weights ----
            a_x = singles.tile([128, KT, M], F8)
            for g in range(NGRP):
                gs = slice(g * GR, (g + 1) * GR)
                for tk in range(KT):
                    nc.sync.dma_start(out=a_x[:, tk, gs], in_=x[:, tk, gs])
                if g == 0:
                    wall0 = singles.tile([128, NT, KT, 128], F8)
                    for tn in range(NT):
                        nc.sync.dma_start(out=wall0[:, tn], in_=w0[tn])
            wall1 = singles.tile([128, NT, KT, 128], F8)
            for tn in range(NT):
                nc.sync.dma_start(out=wall1[:, tn], in_=w1[tn])
            wsl2 = singles.tile([128, KT, 128], BF)
            nc.sync.dma_start(out=wsl2, in_=w2[:, :, :])

            a_h0 = singles.tile([128, NT, M], F8)
            a_h1 = singles.tile([128, NT, M], BF)
            eT = singles.tile([128, M], F32)
            sq = singles.tile([128, M], F32)
            rnorm = singles.tile([128, M], F32)
            fT8 = singles.tile([128, M], F8)
            FT = singles.tile([128, NG], F8)
            dself_all = singles.tile([128, MT], F32)
            dpos_all = singles.tile([128, MT], F32)
            sums_all = singles.tile([128, MT, NGRP * (NCORES // EXPCHUNK)], F32)
            ones = singles.tile([128, 128], F32)
            nc.vector.memset(ones, 1.0)
            nbias = singles.tile([128, 1], F32)
            nc.vector.memset(nbias, -SCALE)
            pbias = singles.tile([128, 1], F32)
            nc.vector.memset(pbias, SCALE)

            def hidden_layer(src, dst, wall, bias_s, func, msl):
                """One fp8 DoubleRow layer for the 512-row slice msl."""
                for tn in range(NT):
                    ps = pmm.tile([128, 512], F32, tag="mm")
                    for tk in range(0, KT, 2):
                        if LDW:
                            nc.tensor.ldweights(
                                weights=wall[:, tn, tk : tk + 2, :],
                                perf_mode=mybir.MatmulPerfMode.DoubleRow,
                            )
                        nc.tensor.matmul(
                            ps,
                            lhsT=wall[:, tn, tk : tk + 2, :],
                            rhs=src[:, tk : tk + 2, msl],
                            start=(tk == 0),
                            stop=(tk == KT - 2),
                            perf_mode=mybir.MatmulPerfMode.DoubleRow,
                        )
                    nc.scalar.activation(
                        out=dst[:, tn, msl], in_=ps, func=func,
                        bias=bias_s[:, tn : tn + 1], scale=1.0,
                    )

            def diag(m, pm, dst):
                """dst[:, m] = diag of fT8_m^T @ fT8_pm (bit-exact fp8 path)."""
                ps_d = psmall.tile([128, 128], F32, tag="ps_small")
                nc.tensor.matmul(
                    ps_d,
                    lhsT=fT8[:, m * 128 : (m + 1) * 128],
                    rhs=fT8[:, pm * 128 : (pm + 1) * 128],
                    start=True, stop=True,
                )
                dsc = small.tile([128, 128], F32, tag="dscratch")
                nc.vector.tensor_mul(dsc, ps_d, ident)
                nc.vector.reduce_sum(
                    dst[:, m : m + 1], dsc, axis=mybir.AxisListType.X
                )

            cc_outs = []
            for g in range(NGRP):
                msl = slice(g * GR, (g + 1) * GR)
                hidden_layer(a_x, a_h0, wall0, b0s, AF.Relu, msl)
                hidden_layer(a_h0, a_h1, wall1, b1s, AF.Identity, msl)

                # layer 2 -> eT[:, msl] fp32
                ps2 = pmm.tile([128, 512], F32, tag="mm")
                for tk in range(KT):
                    nc.tensor.matmul(
                        ps2, lhsT=wsl2[:, tk, :], rhs=a_h1[:, tk, msl],
                        start=(tk == 0), stop=(tk == KT - 1),
                    )
                nc.scalar.activation(
                    out=eT[:, msl], in_=ps2, func=AF.Identity,
                    bias=b2s[:, 0:1], scale=1.0,
                )

                # normalize -> fT8[:, msl]
                nc.vector.tensor_mul(sq[:, msl], eT[:, msl], eT[:, msl])
                psn = pmm.tile([128, 512], F32, tag="mm")
                nc.tensor.matmul(
                    psn, lhsT=ones, rhs=sq[:, msl], start=True, stop=True
                )
                nc.scalar.activation(
                    out=rnorm[:, msl], in_=psn, func=AF.Sqrt, scale=1.0
                )
                nc.vector.reciprocal(out=rnorm[:, msl], in_=rnorm[:, msl])
                nc.vector.tensor_mul(fT8[:, msl], eT[:, msl], rnorm[:, msl])

                # self-similarity diagonals for this group's row tiles
                for m in range(g * MT // NGRP, (g + 1) * MT // NGRP):
                    diag(m, m, dself_all)

                # all-gather this group's normalized features (fp8)
                cc_in = dram.tile([128, GR], F8, name=f"cc_in{g}")
                cc_out = dram.tile(
                    [NCORES * 128, GR], F8, addr_space=CC_ADDR_SPACE,
                    name=f"cc_out{g}",
                )
                nc.sync.dma_start(out=cc_in, in_=fT8[:, msl])
                if "collective" in SKIP:
                    for r in range(NCORES):
                        nc.sync.dma_start(
                            out=cc_out[r * 128 : (r + 1) * 128, :], in_=cc_in[:, :]
                        )
                else:
                    nc.gpsimd.collective_compute(
                        "AllGather",
                        mybir.AluOpType.bypass,
                        replica_groups=[list(range(NCORES))],
                        ins=[cc_in.opt()],
                        outs=[cc_out.opt()],
                    )
                cc_outs.append(cc_out)
                for r in range(NCORES):
                    nc.sync.dma_start(
                        out=FT[:, r * M + g * GR : r * M + (g + 1) * GR],
                        in_=cc_out[r * 128 : (r + 1) * 128, :],
                    )

            # partner-similarity diagonals (pairs cross the two groups)
            for m in range(MT):
                diag(m, (m + MT // 2) % MT, dpos_all)

            # ---- sim + exp-sum: group-0 columns first, then group-1 ----
            outv = singles.tile([128, MT], F32)
            CPG = NCORES // EXPCHUNK  # 1024-col chunks per column group
            for g in range(NGRP):
                for m in range(MT):
                    lhs = fT8[:, m * 128 : (m + 1) * 128]
                    for j in range(CPG):
                        ps = psim.tile([128, EXPCHUNK * 512], F32, tag="mm2")
                        for h in range(EXPCHUNK):
                            r = j * EXPCHUNK + h
                            nc.tensor.matmul(
                                ps[:, h * 512 : (h + 1) * 512], lhsT=lhs,
                                rhs=FT[:, r * M + g * GR : r * M + g * GR + 512],
                                start=True, stop=True,
                            )
                        escr = esc.tile([128, EXPCHUNK * 512], BF, tag="escr")
                        nc.scalar.activation(
                            out=escr, in_=ps, func=AF.Exp, scale=SCALE, bias=nbias,
                        )
                        nc.vector.reduce_sum(
                            sums_all[:, m, g * CPG + j : g * CPG + j + 1],
                            escr, axis=mybir.AxisListType.X,
                        )

            # ---- batched epilogue ----
            stot_all = singles.tile([128, MT], F32)
            for m in range(MT):
                nc.vector.reduce_sum(
                    stot_all[:, m : m + 1], sums_all[:, m, : 2 * CPG],
                    axis=mybir.AxisListType.X,
                )
            eself = small.tile([128, MT], F32, tag="eself")
            nc.scalar.activation(
                out=eself, in_=dself_all, func=AF.Exp, scale=SCALE, bias=nbias
            )
            sexcl = small.tile([128, MT], F32, tag="sexcl")
            nc.vector.tensor_sub(sexcl, stot_all, eself)
            lsep = small.tile([128, MT], F32, tag="lsep")
            nc.scalar.activation(out=lsep, in_=sexcl, func=AF.Ln, scale=1.0)
            post = small.tile([128, MT], F32, tag="post")
            nc.scalar.activation(
                out=post, in_=dpos_all, func=AF.Identity, scale=-SCALE, bias=pbias
            )
            nc.vector.tensor_add(outv, lsep, post)

            nc.sync.dma_start(out=out[:, :], in_=outv)

    nc.finalize()
    return nc


_NC_CACHE = None


def _get_nc():
    global _NC_CACHE
    if _NC_CACHE is None:
        _NC_CACHE = _build()
    return _NC_CACHE


def _prep_w(W, ntiles, dt=BF16):
    K = W.shape[0]
    kt = K // 128
    arr = W.reshape(kt, 128, ntiles, 128).transpose(2, 1, 0, 3)
    return np.ascontiguousarray(arr.astype(dt))


def _prep_b(b, ntiles):
    return np.ascontiguousarray(
        np.asarray(b, np.float32).reshape(ntiles, 128).T
    )


def kernel(input1, input2, W0, b0, W1, b1, W2, b2):
    input1 = np.asarray(input1, np.float32)
    input2 = np.asarray(input2, np.float32)
    w0p = _prep_w(np.asarray(W0, np.float32), NT, FP8)
    w1p = _prep_w(np.asarray(W1, np.float32), NT, FP8)
    w2p = _prep_w(np.asarray(W2, np.float32), 1)[0]
    b0p = _prep_b(b0, NT)
    b1p = _prep_b(b1, NT)
    b2p = np.ascontiguousarray(np.asarray(b2, np.float32).reshape(128, 1))

    in_maps = []
    for r in range(NCORES):
        xr = np.concatenate(
            [input1[r * BS : (r + 1) * BS], input2[r * BS : (r + 1) * BS]], axis=0
        )
        xp = np.ascontiguousarray(
            xr.reshape(M, KT, 128).transpose(2, 1, 0).astype(FP8)
        )
        in_maps.append(
            {
                "x": xp, "w0": w0p, "w1": w1p, "w2": w2p,
                "b0": b0p, "b1": b1p, "b2": b2p,
            }
        )

    nc = _get_nc()
    res = run_bass_kernel_spmd(
        nc,
        in_maps,
        core_ids=list(range(NCORES)),
        trace=bool(int(os.environ.get("KERNEL_TRACE", "0"))),
    )
    total = np.float64(0.0)
    for r in range(NCORES):
        total += np.asarray(res.results[r]["out"], np.float64).sum()
    loss = np.float32(total / (2 * B))
    if res.exec_time_ns is not None:
        kernel.last_exec_time_ns = res.exec_time_ns
    return np.asarray(loss, np.float32)


kernel.last_exec_time_ns = None


# revision 27
# speedup vs baseline: 1.1907x; 1.0058x over previous
"""ContrastiveHead loss kernel for 8 Trainium2 NeuronCores.

Strategy: data-parallel shard B across the 8 cores. Each core runs the
3-layer MLP for its 2*B/8 = 1024 rows in TWO row-groups of 512 so the
all-gather of group 0's normalized fp8 features (Shared-HBM output
collective) overlaps group 1's MLP, and the similarity/exp work for
group-0 columns overlaps the group-1 gather.

Layouts: activations ride transposed ([features-on-partitions, rows-on-
free]); the host pre-transposes the input shard and pre-tiles the
weights into [n_tile][pk, k_tile, jn] slabs. Hidden layers run fp8
DoubleRow; features are normalized then cast to fp8e4m3 for the gather
and the sim matmul (errors in pos cancel against the partner term in
lse, measured rel err ~1e-4).

logsumexp uses the constant bound max=1.0 (normalized rows: sim <= 1):
lse = 1/T + log(sum_j exp((S_ij-1)/T)). The self term is excluded by
subtracting exp((S_ii-1)/T), with S_ii recomputed from the same fp8
bytes the gathered matmul reads, so the cancellation is bit-exact.
"""

import os
import sys

for _p in ("/opt/trn_rl_repo",):
    if os.path.isdir(_p) and _p not in sys.path:
        sys.path.append(_p)

import ml_dtypes
import numpy as np

import concourse.bass as bass
import concourse.mybir as mybir
import concourse.tile as tile
from concourse import bacc
from concourse.bass_utils import run_bass_kernel_spmd
from concourse.masks import make_identity

BF16 = ml_dtypes.bfloat16
F32 = mybir.dt.float32
BF = mybir.dt.bfloat16
F8 = mybir.dt.float8e4
FP8 = mybir.dt.np(F8)

B, D, H, E = 4096, 2048, 2048, 128
T = 0.07
SCALE = float(1.0 / T)
NCORES = 8
BS = B // NCORES          # rows per view per core (512)
M = 2 * BS                # local feature rows (1024)
KT = D // 128             # 16 contraction tiles for D/H
NT = H // 128             # 16 output-feature tiles for hidden layers
MT = M // 128             # 8 local row tiles
NG = NCORES * M           # 8192 gathered rows
NGRP = 2                  # row groups per core (512 rows each)
GR = M // NGRP            # rows per group
SKIP = set(os.environ.get("KERNEL_SKIP", "").split(",")) - {""}
CC_ADDR_SPACE = os.environ.get("KERNEL_CC_ADDR", "Shared")
EXPCHUNK = int(os.environ.get("KERNEL_EXPCHUNK", "1"))
LDW = os.environ.get("KERNEL_LDW", "0") == "1"


def _build():
    nc = bacc.Bacc(num_devices=NCORES)

    x = nc.dram_tensor("x", [128, KT, M], F8, kind="ExternalInput")
    w0 = nc.dram_tensor("w0", [NT, 128, KT, 128], F8, kind="ExternalInput")
    w1 = nc.dram_tensor("w1", [NT, 128, KT, 128], F8, kind="ExternalInput")
    w2 = nc.dram_tensor("w2", [128, KT, 128], BF, kind="ExternalInput")
    b0 = nc.dram_tensor("b0", [128, NT], F32, kind="ExternalInput")
    b1 = nc.dram_tensor("b1", [128, NT], F32, kind="ExternalInput")
    b2 = nc.dram_tensor("b2", [128, 1], F32, kind="ExternalInput")
    out = nc.dram_tensor("out", [128, MT], F32, kind="ExternalOutput")

    AF = mybir.ActivationFunctionType

    with tile.TileContext(nc) as tc:
        with (
            tc.tile_pool(name="singles", bufs=1) as singles,
            tc.tile_pool(name="small", bufs=4) as small,
            tc.tile_pool(name="esc", bufs=3) as esc,
            tc.tile_pool(name="pmm", bufs=2, space="PSUM") as pmm,
            tc.tile_pool(name="psim", bufs=2, space="PSUM") as psim,
            tc.tile_pool(name="psmall", bufs=2, space="PSUM") as psmall,
            tc.tile_pool(name="dram", bufs=1, space="DRAM") as dram,
        ):
            # ---- constants ----
            ident = singles.tile([128, 128], F32)
            make_identity(nc, ident)
            b0s = singles.tile([128, NT], F32)
            b1s = singles.tile([128, NT], F32)
            b2s = singles.tile([128, 1], F32)
            nc.sync.dma_start(out=b0s, in_=b0[:, :])
            nc.sync.dma_start(out=b1s, in_=b1[:, :])
            nc.sync.dma_start(out=b2s, in_=b2[:, :])

            # ---- activations + resident weights ----
            a_x = singles.tile([128, KT, M], F8)
            for g in range(NGRP):
                gs = slice(g * GR, (g + 1) * GR)
                for tk in range(KT):
                    nc.sync.dma_start(out=a_x[:, tk, gs], in_=x[:, tk, gs])
                if g == 0:
                    wall0 = singles.tile([128, NT, KT, 128], F8)
                    for tn in range(NT):
                        nc.sync.dma_start(out=wall0[:, tn], in_=w0[tn])
            wall1 = singles.tile([128, NT, KT, 128], F8)
            for tn in range(NT):
                nc.sync.dma_start(out=wall1[:, tn], in_=w1[tn])
            wsl2 = singles.tile([128, KT, 128], BF)
            nc.sync.dma_start(out=wsl2, in_=w2[:, :, :])

            a_h0 = singles.tile([128, NT, M], F8)
            a_h1 = singles.tile([128, NT, M], BF)
            eT = singles.tile([128, M], F32)
            sq = singles.tile([128, M], F32)
            rnorm = singles.tile([128, M], F32)
            fT8 = singles.tile([128, M], F8)
            FT = singles.tile([128, NG], F8)
            dself_all = singles.tile([128, MT], F32)
            dpos_all = singles.tile([128, MT], F32)
            sums_all = singles.tile([128, MT, NGRP * (NCORES // EXPCHUNK)], F32)
            ones = singles.tile([128, 128], F32)
            nc.vector.memset(ones, 1.0)
            nbias = singles.tile([128, 1], F32)
            nc.vector.memset(nbias, -SCALE)
            pbias = singles.tile([128, 1], F32)
            nc.vector.memset(pbias, SCALE)

            def hidden_layer(src, dst, wall, bias_s, func, msl):
                """One fp8 DoubleRow layer for the 512-row slice msl."""
                for tn in range(NT):
                    ps = pmm.tile([128, 512], F32, tag="mm")
                    for tk in range(0, KT, 2):
                        if LDW:
                            nc.tensor.ldweights(
                                weights=wall[:, tn, tk : tk + 2, :],
                                perf_mode=mybir.MatmulPerfMode.DoubleRow,
                            )
                        nc.tensor.matmul(
                            ps,
                            lhsT=wall[:, tn, tk : tk + 2, :],
                            rhs=src[:, tk : tk + 2, msl],
                            start=(tk == 0),
                            stop=(tk == KT - 2),
                            perf_mode=mybir.MatmulPerfMode.DoubleRow,
                        )
                    nc.scalar.activation(
                        out=dst[:, tn, msl], in_=ps, func=func,
                        bias=bias_s[:, tn : tn + 1], scale=1.0,
                    )

            def diag(m, pm, dst):
                """dst[:, m] = diag of fT8_m^T @ fT8_pm (bit-exact fp8 path)."""
                ps_d = psmall.tile([128, 128], F32, tag="ps_small")
                nc.tensor.matmul(
                    ps_d,
                    lhsT=fT8[:, m * 128 : (m + 1) * 128],
                    rhs=fT8[:, pm * 128 : (pm + 1) * 128],
                    start=True, stop=True,
                )
                dsc = small.tile([128, 128], F32, tag="dscratch")
                nc.vector.tensor_mul(dsc, ps_d, ident)
                nc.vector.reduce_sum(
                    dst[:, m : m + 1], dsc, axis=mybir.AxisListType.X
                )

            cc_outs = []
            for g in range(NGRP):
                msl = slice(g * GR, (g + 1) * GR)
                hidden_layer(a_x, a_h0, wall0, b0s, AF.Relu, msl)
                hidden_layer(a_h0, a_h1, wall1, b1s, AF.Identity, msl)

                # layer 2 -> eT[:, msl] fp32
                ps2 = pmm.tile([128, 512], F32, tag="mm")
                for tk in range(KT):
                    nc.tensor.matmul(
                        ps2, lhsT=wsl2[:, tk, :], rhs=a_h1[:, tk, msl],
                        start=(tk == 0), stop=(tk == KT - 1),
                    )
                nc.scalar.activation(
                    out=eT[:, msl], in_=ps2, func=AF.Identity,
                    bias=b2s[:, 0:1], scale=1.0,
                )

                # normalize -> fT8[:, msl] (rsqrt on scalar: short critical
                # path to the gather; table error cancels between lse and pos)
                nc.vector.tensor_mul(sq[:, msl], eT[:, msl], eT[:, msl])
                psn = pmm.tile([128, 512], F32, tag="mm")
                nc.tensor.matmul(
                    psn, lhsT=ones, rhs=sq[:, msl], start=True, stop=True
                )
                nc.scalar.activation(
                    out=rnorm[:, msl], in_=psn, func=AF.Sqrt, scale=1.0
                )
                nc.vector.reciprocal(out=rnorm[:, msl], in_=rnorm[:, msl])
                nc.vector.tensor_mul(fT8[:, msl], eT[:, msl], rnorm[:, msl])

                # all-gather this group's normalized features (fp8)
                cc_in = dram.tile([128, GR], F8, name=f"cc_in{g}")
                cc_out = dram.tile(
                    [NCORES * 128, GR], F8, addr_space=CC_ADDR_SPACE,
                    name=f"cc_out{g}",
                )
                nc.sync.dma_start(out=cc_in, in_=fT8[:, msl])
                if "collective" in SKIP:
                    for r in range(NCORES):
                        nc.sync.dma_start(
                            out=cc_out[r * 128 : (r + 1) * 128, :], in_=cc_in[:, :]
                        )
                else:
                    nc.gpsimd.collective_compute(
                        "AllGather",
                        mybir.AluOpType.bypass,
                        replica_groups=[list(range(NCORES))],
                        ins=[cc_in.opt()],
                        outs=[cc_out.opt()],
                    )
                cc_outs.append(cc_out)
                for r in range(NCORES):
                    nc.sync.dma_start(
                        out=FT[:, r * M + g * GR : r * M + (g + 1) * GR],
                        in_=cc_out[r * 128 : (r + 1) * 128, :],
                    )

                # self-similarity diagonals (off the gather critical path)
                for m in range(g * MT // NGRP, (g + 1) * MT // NGRP):
                    diag(m, m, dself_all)

            # partner-similarity diagonals (pairs cross the two groups)
            for m in range(MT):
                diag(m, (m + MT // 2) % MT, dpos_all)

            # ---- sim + exp-sum: group-0 columns first, then group-1 ----
            outv = singles.tile([128, MT], F32)
            CPG = NCORES // EXPCHUNK  # 1024-col chunks per column group
            for g in range(NGRP):
                for m in range(MT):
                    lhs = fT8[:, m * 128 : (m + 1) * 128]
                    for j in range(CPG):
                        ps = psim.tile([128, EXPCHUNK * 512], F32, tag="mm2")
                        for h in range(EXPCHUNK):
                            r = j * EXPCHUNK + h
                            nc.tensor.matmul(
                                ps[:, h * 512 : (h + 1) * 512], lhsT=lhs,
                                rhs=FT[:, r * M + g * GR : r * M + g * GR + 512],
                                start=True, stop=True,
                            )
                        escr = esc.tile([128, EXPCHUNK * 512], BF, tag="escr")
                        nc.scalar.activation(
                            out=escr, in_=ps, func=AF.Exp, scale=SCALE, bias=nbias,
                        )
                        nc.vector.reduce_sum(
                            sums_all[:, m, g * CPG + j : g * CPG + j + 1],
                            escr, axis=mybir.AxisListType.X,
                        )

            # ---- batched epilogue ----
            stot_all = singles.tile([128, MT], F32)
            for m in range(MT):
                nc.vector.reduce_sum(
                    stot_all[:, m : m + 1], sums_all[:, m, : 2 * CPG],
                    axis=mybir.AxisListType.X,
                )
            eself = small.tile([128, MT], F32, tag="eself")
            nc.scalar.activation(
                out=eself, in_=dself_all, func=AF.Exp, scale=SCALE, bias=nbias
            )
            sexcl = small.tile([128, MT], F32, tag="sexcl")
            nc.vector.tensor_sub(sexcl, stot_all, eself)
            lsep = small.tile([128, MT], F32, tag="lsep")
            nc.scalar.activation(out=lsep, in_=sexcl, func=AF.Ln, scale=1.0)
            post = small.tile([128, MT], F32, tag="post")
            nc.scalar.activation(
                out=post, in_=dpos_all, func=AF.Identity, scale=-SCALE, bias=pbias
            )
            nc.vector.tensor_add(outv, lsep, post)

            nc.sync.dma_start(out=out[:, :], in_=outv)

    nc.finalize()
    return nc


_NC_CACHE = None


def _get_nc():
    global _NC_CACHE
    if _NC_CACHE is None:
        _NC_CACHE = _build()
    return _NC_CACHE


def _prep_w(W, ntiles, dt=BF16):
    K = W.shape[0]
    kt = K // 128
    arr = W.reshape(kt, 128, ntiles, 128).transpose(2, 1, 0, 3)
    return np.ascontiguousarray(arr.astype(dt))


def _prep_b(b, ntiles):
    return np.ascontiguousarray(
        np.asarray(b, np.float32).reshape(ntiles, 128).T
    )


def kernel(input1, input2, W0, b0, W1, b1, W2, b2):
    input1 = np.asarray(input1, np.float32)
    input2 = np.asarray(input2, np.float32)
    w0p = _prep_w(np.asarray(W0, np.float32), NT, FP8)
    w1p = _prep_w(np.asarray(W1, np.float32), NT, FP8)
    w2p = _prep_w(np.asarray(W2, np.float32), 1)[0]
    b0p = _prep_b(b0, NT)
    b1p = _prep_b(b1, NT)
    b2p = np.ascontiguousarray(np.asarray(b2, np.float32).reshape(128, 1))

    in_maps = []
    for r in range(NCORES):
        xr = np.concatenate(
            [input1[r * BS : (r + 1) * BS], input2[r * BS : (r + 1) * BS]], axis=0
        )
        xp = np.ascontiguousarray(
            xr.reshape(M, KT, 128).transpose(2, 1, 0).astype(FP8)
        )
        in_maps.append(
            {
                "x": xp, "w0": w0p, "w1": w1p, "w2": w2p,
                "b0": b0p, "b1": b1p, "b2": b2p,
            }
        )

    nc = _get_nc()
    res = run_bass_kernel_spmd(
        nc,
        in_maps,
        core_ids=list(range(NCORES)),
        trace=bool(int(os.environ.get("KERNEL_TRACE", "0"))),
    )
    total = np.float64(0.0)
    for r in range(NCORES):
        total += np.asarray(res.results[r]["out"], np.float64).sum()
    loss = np.float32(total / (2 * B))
    if res.exec_time_ns is not None:
        kernel.last_exec_time_ns = res.exec_time_ns
    return np.asarray(loss, np.float32)


kernel.last_exec_time_ns = None


# revision 30
# speedup vs baseline: 1.2037x; 1.0109x over previous
"""ContrastiveHead loss kernel for 8 Trainium2 NeuronCores.

Strategy: data-parallel shard B across the 8 cores. Each core runs the
3-layer MLP for its 2*B/8 = 1024 rows in TWO row-groups of 512 so the
all-gather of group 0's normalized fp8 features (Shared-HBM output
collective) overlaps group 1's MLP, and the similarity/exp work for
group-0 columns overlaps the group-1 gather.

Layouts: activations ride transposed ([features-on-partitions, rows-on-
free]); the host pre-transposes the input shard and pre-tiles the
weights into [n_tile][pk, k_tile, jn] slabs. Hidden layers run fp8
DoubleRow; features are normalized then cast to fp8e4m3 for the gather
and the sim matmul (errors in pos cancel against the partner term in
lse, measured rel err ~1e-4).

logsumexp uses the constant bound max=1.0 (normalized rows: sim <= 1):
lse = 1/T + log(sum_j exp((S_ij-1)/T)). The self term is excluded by
subtracting exp((S_ii-1)/T), with S_ii recomputed from the same fp8
bytes the gathered matmul reads, so the cancellation is bit-exact.
"""

import os
import sys

for _p in ("/opt/trn_rl_repo",):
    if os.path.isdir(_p) and _p not in sys.path:
        sys.path.append(_p)

import ml_dtypes
import numpy as np

import concourse.bass as bass
import concourse.mybir as mybir
import concourse.tile as tile
from concourse import bacc
from concourse.bass_utils import run_bass_kernel_spmd
from concourse.masks import make_identity

BF16 = ml_dtypes.bfloat16
F32 = mybir.dt.float32
BF = mybir.dt.bfloat16
F8 = mybir.dt.float8e4
FP8 = mybir.dt.np(F8)

B, D, H, E = 4096, 2048, 2048, 128
T = 0.07
SCALE = float(1.0 / T)
NCORES = 8
BS = B // NCORES          # rows per view per core (512)
M = 2 * BS                # local feature rows (1024)
KT = D // 128             # 16 contraction tiles for D/H
NT = H // 128             # 16 output-feature tiles for hidden layers
MT = M // 128             # 8 local row tiles
NG = NCORES * M           # 8192 gathered rows
NGRP = 2                  # row groups per core (512 rows each)
GR = M // NGRP            # rows per group
SKIP = set(os.environ.get("KERNEL_SKIP", "").split(",")) - {""}
CC_ADDR_SPACE = os.environ.get("KERNEL_CC_ADDR", "Shared")
EXPCHUNK = int(os.environ.get("KERNEL_EXPCHUNK", "1"))
LDW = os.environ.get("KERNEL_LDW", "0") == "1"


def _build():
    nc = bacc.Bacc(num_devices=NCORES)

    x = nc.dram_tensor("x", [128, KT, M], F8, kind="ExternalInput")
    w0 = nc.dram_tensor("w0", [NT, 128, KT, 128], F8, kind="ExternalInput")
    w1 = nc.dram_tensor("w1", [NT, 128, KT, 128], F8, kind="ExternalInput")
    w2 = nc.dram_tensor("w2", [128, KT, 128], BF, kind="ExternalInput")
    b0 = nc.dram_tensor("b0", [128, NT], F32, kind="ExternalInput")
    b1 = nc.dram_tensor("b1", [128, NT], F32, kind="ExternalInput")
    b2 = nc.dram_tensor("b2", [128, 1], F32, kind="ExternalInput")
    out = nc.dram_tensor("out", [128, MT], F32, kind="ExternalOutput")

    AF = mybir.ActivationFunctionType

    with tile.TileContext(nc) as tc:
        with (
            tc.tile_pool(name="singles", bufs=1) as singles,
            tc.tile_pool(name="small", bufs=4) as small,
            tc.tile_pool(name="esc", bufs=3) as esc,
            tc.tile_pool(name="pmm", bufs=2, space="PSUM") as pmm,
            tc.tile_pool(name="psim", bufs=2, space="PSUM") as psim,
            tc.tile_pool(name="psmall", bufs=2, space="PSUM") as psmall,
            tc.tile_pool(name="dram", bufs=1, space="DRAM") as dram,
        ):
            # ---- constants ----
            ident = singles.tile([128, 128], F32)
            make_identity(nc, ident)
            b0s = singles.tile([128, NT], F32)
            b1s = singles.tile([128, NT], F32)
            b2s = singles.tile([128, 1], F32)
            nc.sync.dma_start(out=b0s, in_=b0[:, :])
            nc.sync.dma_start(out=b1s, in_=b1[:, :])
            nc.sync.dma_start(out=b2s, in_=b2[:, :])

            # ---- activations + resident weights ----
            a_x = singles.tile([128, KT, M], F8)
            for g in range(NGRP):
                gs = slice(g * GR, (g + 1) * GR)
                for tk in range(KT):
                    nc.sync.dma_start(out=a_x[:, tk, gs], in_=x[:, tk, gs])
                if g == 0:
                    wall0 = singles.tile([128, NT, KT, 128], F8)
                    for tn in range(NT):
                        nc.sync.dma_start(out=wall0[:, tn], in_=w0[tn])
            wall1 = singles.tile([128, NT, KT, 128], F8)
            for tn in range(NT):
                nc.sync.dma_start(out=wall1[:, tn], in_=w1[tn])
            wsl2 = singles.tile([128, KT, 128], BF)
            nc.sync.dma_start(out=wsl2, in_=w2[:, :, :])

            a_h0 = singles.tile([128, NT, M], F8)
            a_h1 = singles.tile([128, NT, M], BF)
            eT = singles.tile([128, M], F32)
            sq = singles.tile([128, M], F32)
            rnorm = singles.tile([128, M], F32)
            fT8 = singles.tile([128, M], F8)
            FT = singles.tile([128, NG], F8)
            dself_all = singles.tile([128, MT], F32)
            dpos_all = singles.tile([128, MT], F32)
            sums_all = singles.tile([128, MT, NGRP * (NCORES // EXPCHUNK)], F32)
            ones = singles.tile([128, 128], F32)
            nc.vector.memset(ones, 1.0)
            nbias = singles.tile([128, 1], F32)
            nc.vector.memset(nbias, -SCALE)
            pbias = singles.tile([128, 1], F32)
            nc.vector.memset(pbias, SCALE)

            CPG = NCORES // EXPCHUNK  # 512*EXPCHUNK-col chunks per col group

            def sim_chunk(g, m, j):
                """exp-sum of sim block (row tile m) x (col chunk j of group g)."""
                lhs = fT8[:, m * 128 : (m + 1) * 128]
                ps = psim.tile([128, EXPCHUNK * 512], F32, tag="mm2")
                for h in range(EXPCHUNK):
                    r = j * EXPCHUNK + h
                    nc.tensor.matmul(
                        ps[:, h * 512 : (h + 1) * 512], lhsT=lhs,
                        rhs=FT[:, r * M + g * GR : r * M + g * GR + 512],
                        start=True, stop=True,
                    )
                escr = esc.tile([128, EXPCHUNK * 512], BF, tag="escr")
                nc.scalar.activation(
                    out=escr, in_=ps, func=AF.Exp, scale=SCALE, bias=nbias,
                )
                nc.vector.reduce_sum(
                    sums_all[:, m, g * CPG + j : g * CPG + j + 1],
                    escr, axis=mybir.AxisListType.X,
                )

            def hidden_layer(src, dst, wall, bias_s, func, msl, inject=None):
                """One fp8 DoubleRow layer for the 512-row slice msl."""
                for tn in range(NT):
                    ps = pmm.tile([128, 512], F32, tag="mm")
                    for tk in range(0, KT, 2):
                        if LDW:
                            nc.tensor.ldweights(
                                weights=wall[:, tn, tk : tk + 2, :],
                                perf_mode=mybir.MatmulPerfMode.DoubleRow,
                            )
                        nc.tensor.matmul(
                            ps,
                            lhsT=wall[:, tn, tk : tk + 2, :],
                            rhs=src[:, tk : tk + 2, msl],
                            start=(tk == 0),
                            stop=(tk == KT - 2),
                            perf_mode=mybir.MatmulPerfMode.DoubleRow,
                        )
                    nc.scalar.activation(
                        out=dst[:, tn, msl], in_=ps, func=func,
                        bias=bias_s[:, tn : tn + 1], scale=1.0,
                    )
                    if inject is not None:
                        inject(tn)

            def diag(m, pm, dst):
                """dst[:, m] = diag of fT8_m^T @ fT8_pm (bit-exact fp8 path)."""
                ps_d = psmall.tile([128, 128], F32, tag="ps_small")
                nc.tensor.matmul(
                    ps_d,
                    lhsT=fT8[:, m * 128 : (m + 1) * 128],
                    rhs=fT8[:, pm * 128 : (pm + 1) * 128],
                    start=True, stop=True,
                )
                dsc = small.tile([128, 128], F32, tag="dscratch")
                nc.vector.tensor_mul(dsc, ps_d, ident)
                nc.vector.reduce_sum(
                    dst[:, m : m + 1], dsc, axis=mybir.AxisListType.X
                )

            # sim chunks for (row tiles 0-3) x (group-0 columns): available
            # once gather 0 lands, so they interleave into group 1's L1 loop
            # (Identity acts keep the EXP table set resident).
            ICHUNKS = [(m, j) for m in range(MT // NGRP) for j in range(CPG)]
            PER_TN = len(ICHUNKS) // 8

            def inject_l1(tn):
                if tn >= 8:
                    for mm, jj in ICHUNKS[(tn - 8) * PER_TN : (tn - 7) * PER_TN]:
                        sim_chunk(0, mm, jj)

            cc_outs = []
            for g in range(NGRP):
                msl = slice(g * GR, (g + 1) * GR)
                hidden_layer(a_x, a_h0, wall0, b0s, AF.Relu, msl)
                hidden_layer(
                    a_h0, a_h1, wall1, b1s, AF.Identity, msl,
                    inject=inject_l1 if g == 1 else None,
                )

                # layer 2 -> eT[:, msl] fp32
                ps2 = pmm.tile([128, 512], F32, tag="mm")
                for tk in range(KT):
                    nc.tensor.matmul(
                        ps2, lhsT=wsl2[:, tk, :], rhs=a_h1[:, tk, msl],
                        start=(tk == 0), stop=(tk == KT - 1),
                    )
                nc.scalar.activation(
                    out=eT[:, msl], in_=ps2, func=AF.Identity,
                    bias=b2s[:, 0:1], scale=1.0,
                )

                # normalize -> fT8[:, msl] (rsqrt on scalar: short critical
                # path to the gather; table error cancels between lse and pos)
                nc.vector.tensor_mul(sq[:, msl], eT[:, msl], eT[:, msl])
                psn = pmm.tile([128, 512], F32, tag="mm")
                nc.tensor.matmul(
                    psn, lhsT=ones, rhs=sq[:, msl], start=True, stop=True
                )
                nc.scalar.activation(
                    out=rnorm[:, msl], in_=psn, func=AF.Sqrt, scale=1.0
                )
                nc.vector.reciprocal(out=rnorm[:, msl], in_=rnorm[:, msl])
                nc.vector.tensor_mul(fT8[:, msl], eT[:, msl], rnorm[:, msl])

                # all-gather this group's normalized features (fp8)
                cc_in = dram.tile([128, GR], F8, name=f"cc_in{g}")
                cc_out = dram.tile(
                    [NCORES * 128, GR], F8, addr_space=CC_ADDR_SPACE,
                    name=f"cc_out{g}",
                )
                nc.sync.dma_start(out=cc_in, in_=fT8[:, msl])
                if "collective" in SKIP:
                    for r in range(NCORES):
                        nc.sync.dma_start(
                            out=cc_out[r * 128 : (r + 1) * 128, :], in_=cc_in[:, :]
                        )
                else:
                    nc.gpsimd.collective_compute(
                        "AllGather",
                        mybir.AluOpType.bypass,
                        replica_groups=[list(range(NCORES))],
                        ins=[cc_in.opt()],
                        outs=[cc_out.opt()],
                    )
                cc_outs.append(cc_out)
                for r in range(NCORES):
                    nc.sync.dma_start(
                        out=FT[:, r * M + g * GR : r * M + (g + 1) * GR],
                        in_=cc_out[r * 128 : (r + 1) * 128, :],
                    )

                # self-similarity diagonals (off the gather critical path)
                for m in range(g * MT // NGRP, (g + 1) * MT // NGRP):
                    diag(m, m, dself_all)

            # partner-similarity diagonals (pairs cross the two groups)
            for m in range(MT):
                diag(m, (m + MT // 2) % MT, dpos_all)

            # ---- sim + exp-sum: group-0 columns first, then group-1 ----
            outv = singles.tile([128, MT], F32)
            for g in range(NGRP):
                for m in range(MT):
                    if g == 0 and m < MT // NGRP:
                        continue  # interleaved into group 1's L1 above
                    for j in range(CPG):
                        sim_chunk(g, m, j)

            # ---- batched epilogue ----
            stot_all = singles.tile([128, MT], F32)
            for m in range(MT):
                nc.vector.reduce_sum(
                    stot_all[:, m : m + 1], sums_all[:, m, : 2 * CPG],
                    axis=mybir.AxisListType.X,
                )
            eself = small.tile([128, MT], F32, tag="eself")
            nc.scalar.activation(
                out=eself, in_=dself_all, func=AF.Exp, scale=SCALE, bias=nbias
            )
            sexcl = small.tile([128, MT], F32, tag="sexcl")
            nc.vector.tensor_sub(sexcl, stot_all, eself)
            lsep = small.tile([128, MT], F32, tag="lsep")
            nc.scalar.activation(out=lsep, in_=sexcl, func=AF.Ln, scale=1.0)
            post = small.tile([128, MT], F32, tag="post")
            nc.scalar.activation(
                out=post, in_=dpos_all, func=AF.Identity, scale=-SCALE, bias=pbias
            )
            nc.vector.tensor_add(outv, lsep, post)

            nc.sync.dma_start(out=out[:, :], in_=outv)

    nc.finalize()
    return nc


_NC_CACHE = None


def _get_nc():
    global _NC_CACHE
    if _NC_CACHE is None:
        _NC_CACHE = _build()
    return _NC_CACHE


def _prep_w(W, ntiles, dt=BF16):
    K = W.shape[0]
    kt = K // 128
    arr = W.reshape(kt, 128, ntiles, 128).transpose(2, 1, 0, 3)
    return np.ascontiguousarray(arr.astype(dt))


def _prep_b(b, ntiles):
    return np.ascontiguousarray(
        np.asarray(b, np.float32).reshape(ntiles, 128).T
    )


def kernel(input1, input2, W0, b0, W1, b1, W2, b2):
    input1 = np.asarray(input1, np.float32)
    input2 = np.asarray(input2, np.float32)
    w0p = _prep_w(np.asarray(W0, np.float32), NT, FP8)
    w1p = _prep_w(np.asarray(W1, np.float32), NT, FP8)
    w2p = _prep_w(np.asarray(W2, np.float32), 1)[0]
    b0p = _prep_b(b0, NT)
    b1p = _prep_b(b1, NT)
    b2p = np.ascontiguousarray(np.asarray(b2, np.float32).reshape(128, 1))

    in_maps = []
    for r in range(NCORES):
        xr = np.concatenate(
            [input1[r * BS : (r + 1) * BS], input2[r * BS : (r + 1) * BS]], axis=0
        )
        xp = np.ascontiguousarray(
            xr.reshape(M, KT, 128).transpose(2, 1, 0).astype(FP8)
        )
        in_maps.append(
            {
                "x": xp, "w0": w0p, "w1": w1p, "w2": w2p,
                "b0": b0p, "b1": b1p, "b2": b2p,
            }
        )

    nc = _get_nc()
    res = run_bass_kernel_spmd(
        nc,
        in_maps,
        core_ids=list(range(NCORES)),
        trace=bool(int(os.environ.get("KERNEL_TRACE", "0"))),
    )
    total = np.float64(0.0)
    for r in range(NCORES):
        total += np.asarray(res.results[r]["out"], np.float64).sum()
    loss = np.float32(total / (2 * B))
    if res.exec_time_ns is not None:
        kernel.last_exec_time_ns = res.exec_time_ns
    return np.asarray(loss, np.float32)


kernel.last_exec_time_ns = None
